# revision 17
# baseline (speedup 1.0000x reference)
"""Self-contained Trainium2 Bass kernel for nn_CPINet_36850819400255.

Strategy: pure data parallelism over batch B=256 -> 8 cores x 32 samples.
Per core the dominant cost is the 3-layer 23x23 conv over [2048, 64] maps,
computed as 12 accumulating K=128 fp16 matmuls per 512-col block against a
transposed, zero-padded image whose partition rows 64..127 hold a copy
shifted by one column (each matmul covers two kernel rows).  Two samples
run concurrently in PE column groups 0-63 / 64-127.  Embedding gathers and
the layer-0 image transpose are host-side input transforms; the device
sees ready-made [128, 2080] fp16 images.  GNN keeps its state transposed:
px^T = (A @ hs)^T is computed as lhsT=hs, rhs=A^T (no PE transposes).
"""

import sys

sys.path.insert(0, "/opt/trn_rl_repo")

import numpy as np

import concourse.bass as bass
import concourse.mybir as mybir
import concourse.tile as tile
from concourse import bacc
from concourse.bass_utils import run_bass_kernel_spmd

F32 = mybir.dt.float32
F16 = mybir.dt.float16
AF = mybir.ActivationFunctionType
OP = mybir.AluOpType

NCORES = 8
B_TOT = 256
NS = B_TOT // NCORES          # samples per core
N = 128                       # atoms
L = 2048                      # amino length
D = 64
LP = 16                       # left pad of the conv image (32B aligned)
XW = 2080                     # padded width of transposed conv image
EPS = 1e-6


def build_nc(nsamp=NS, stage=99, sub=99):
    """stage: 1=prep only, 2=+conv, 3=+attention, 99=full."""
    nc = bacc.Bacc("TRN2", target_bir_lowering=False, debug=True)

    # ---- DRAM I/O ----
    ximg_d = nc.dram_tensor("ximg", [nsamp, 128, XW], F16, kind="ExternalInput")
    xst0_d = nc.dram_tensor("xst0", [nsamp, D + 1, N], F32, kind="ExternalInput")
    amask_d = nc.dram_tensor("amask", [nsamp, N], F32, kind="ExternalInput")
    pmask_d = nc.dram_tensor("pmask", [nsamp, L], F32, kind="ExternalInput")
    adjt_d = nc.dram_tensor("adjt", [nsamp, N, N], F16, kind="ExternalInput")
    wg_d = nc.dram_tensor("wg", [3, D + 1, D], F16, kind="ExternalInput")
    tk_d = nc.dram_tensor("tk", [3, 12, 128, D], F16, kind="ExternalInput")
    cb_d = nc.dram_tensor("cb", [3, 128], F32, kind="ExternalInput")
    wa_d = nc.dram_tensor("wa", [128, D], F16, kind="ExternalInput")
    ba_d = nc.dram_tensor("ba", [128], F32, kind="ExternalInput")
    wo_d = nc.dram_tensor("wo", [2, 128, 128], F32, kind="ExternalInput")
    bo_d = nc.dram_tensor("bo", [2, 128], F32, kind="ExternalInput")
    wi_d = nc.dram_tensor("wi", [128, 2], F32, kind="ExternalInput")
    bi_d = nc.dram_tensor("bi", [2], F32, kind="ExternalInput")
    out_d = nc.dram_tensor("out", [2, nsamp], F32, kind="ExternalOutput")

    with tile.TileContext(nc) as tc:
        with (
            tc.tile_pool(name="cp", bufs=1) as cp,          # constants
            tc.tile_pool(name="xp", bufs=10) as xp,         # conv images
            tc.tile_pool(name="pp", bufs=4) as pp,          # psT / hsT
            tc.tile_pool(name="sm", bufs=3) as sm,          # small sbuf
            tc.tile_pool(name="pc", bufs=4, space="PSUM") as pc,   # conv psum
            tc.tile_pool(name="pa", bufs=2, space="PSUM") as pa,   # attn psum
            tc.tile_pool(name="pz", bufs=2, space="PSUM") as pz,   # small psum
        ):
            # ---------- constants ----------
            ones_c = cp.tile([128, D], F32, tag="ones_c")
            nc.vector.memset(ones_c[:], 1.0)
            ones65 = cp.tile([1, D + 1], F32, tag="ones65")
            nc.vector.memset(ones65[:], 1.0)
            ones_rh = cp.tile([1, D], F16, tag="ones_rh")
            nc.vector.memset(ones_rh[:], 1.0)

            tk_sb = cp.tile([128, 3 * 12 * D], F16, tag="tk")
            for i in range(3):
                for j in range(12):
                    k = i * 12 + j
                    nc.sync.dma_start(tk_sb[:, k * D:(k + 1) * D], tk_d[i, j])
            wg_sb = cp.tile([D + 1, 3 * D], F16, tag="wg")
            for i in range(3):
                nc.sync.dma_start(wg_sb[:, i * D:(i + 1) * D], wg_d[i])
            wa_sb = cp.tile([128, D], F16, tag="wa")
            nc.sync.dma_start(wa_sb[:], wa_d[:])
            ba_sb = cp.tile([128, 1], F32, tag="ba")
            nc.sync.dma_start(ba_sb[:], ba_d[:, None])
            cb_sb = cp.tile([128, 3], F32, tag="cb")
            for i in range(3):
                nc.sync.dma_start(cb_sb[:, i:i + 1], cb_d[i, :, None])
            wo_sb = cp.tile([128, 256], F32, tag="wo")
            for j in range(2):
                nc.sync.dma_start(wo_sb[:, j * 128:(j + 1) * 128], wo_d[j])
            bo_sb = cp.tile([128, 2], F32, tag="bo")
            for j in range(2):
                nc.sync.dma_start(bo_sb[:, j:j + 1], bo_d[j, :, None])
            wi_sb = cp.tile([128, 2], F32, tag="wi")
            nc.sync.dma_start(wi_sb[:], wi_d[:])
            bi_sb = cp.tile([2, 1], F32, tag="bi")
            nc.sync.dma_start(bi_sb[:], bi_d[:, None])

            catC = cp.tile([128, nsamp], F32, tag="cat")

            def prep_sample(s):
                """DMA loads + GNN + compound for sample s.
                Returns (X, cTb, prc, pm_row)."""
                X = xp.tile([128, XW], F16, tag="X")
                nc.sync.dma_start(X[:], ximg_d[s])

                am_col = sm.tile([N, 1], F32, tag="amcol")
                nc.sync.dma_start(am_col[:], amask_d[s, :, None])
                am_row = sm.tile([1, N], F32, tag="amrow")
                nc.sync.dma_start(am_row[:], amask_d[s, None, :])
                ATh = sm.tile([N, N], F16, tag="AT")
                nc.sync.dma_start(ATh[:], adjt_d[s])

                # ---- GNN (state kept transposed: xsT [65, 128] fp32) ----
                xsT = sm.tile([D + 1, N], F32, tag="xst")
                nc.sync.dma_start(xsT[:], xst0_d[s])
                for i in range(3):
                    xsTb = sm.tile([D + 1, N], F16, tag="xstb")
                    nc.vector.tensor_copy(xsTb[:], xsT[:])
                    ph = pz.tile([128, 512], F32, tag="ss")
                    nc.tensor.matmul(ph[0:N, 0:D], xsTb[:],
                                     wg_sb[:, i * D:(i + 1) * D],
                                     start=True, stop=True)
                    hs = sm.tile([N, D], F16, tag="hs")
                    nc.scalar.activation(hs[:], ph[0:N, 0:D], AF.Relu)
                    px = pz.tile([128, 512], F32, tag="ss")
                    nc.tensor.matmul(px[0:D, 0:N], hs[:], ATh[:],
                                     start=True, stop=True)
                    xsT2 = sm.tile([D + 1, N], F32, tag="xst")
                    nc.vector.memset(xsT2[D:D + 1, :], 1.0)
                    nc.vector.tensor_add(xsT2[0:D, :], px[0:D, 0:N], xsT[0:D, :])
                    xsT = xsT2

                # ---- compound (masked mean over atoms) ----
                amb = pz.tile([128, 512], F32, tag="ss")
                nc.tensor.matmul(amb[0:D + 1, 0:N], ones65[:], am_row[:],
                                 start=True, stop=True)
                scrc = sm.tile([D, N], F32, tag="scrc")
                cm = sm.tile([D, 1], F32, tag="cm")
                nc.vector.scalar_tensor_tensor(
                    scrc[:], xsT[0:D, :], 1.0, amb[0:D, 0:N],
                    op0=OP.mult, op1=OP.mult, accum_out=cm[:])
                pd = pz.tile([128, 512], F32, tag="ss")
                nc.tensor.matmul(pd[0:D, 0:1], ones_c[:], am_col[:],
                                 start=True, stop=True)
                dn = sm.tile([D, 1], F32, tag="dn")
                nc.vector.tensor_scalar_add(dn[:], pd[0:D, 0:1], EPS)
                rc = sm.tile([D, 1], F32, tag="rc")
                nc.vector.reciprocal(rc[:], dn[:])
                cT = sm.tile([D, 1], F32, tag="ct")
                nc.vector.tensor_tensor(cT[:], cm[:], rc[:], op=OP.mult)
                nc.vector.tensor_copy(catC[0:D, s:s + 1], cT[:])
                cTb = sm.tile([D, 1], F16, tag="ctb")
                nc.vector.tensor_copy(cTb[:], cT[:])

                # ---- protein mask ----
                pm16 = sm.tile([128, 16], F32, tag="pm16")
                nc.sync.dma_start(pm16[:], pmask_d[s].rearrange("(p t) -> p t", t=16))
                pmj = sm.tile([128, 16], F32, tag="pmj")
                pmsum = sm.tile([128, 1], F32, tag="pmsum")
                nc.scalar.activation(pmj[:], pm16[:], AF.Copy, accum_out=pmsum[:])
                ppd = pz.tile([128, 512], F32, tag="ss")
                nc.tensor.matmul(ppd[0:D, 0:1], ones_c[:], pmsum[:], start=True, stop=True)
                pdn = sm.tile([D, 1], F32, tag="pdn")
                nc.vector.tensor_scalar_add(pdn[:], ppd[0:D, 0:1], EPS)
                prc = sm.tile([D, 1], F32, tag="prc")
                nc.vector.reciprocal(prc[:], pdn[:])
                pm_row = sm.tile([1, L], F32, tag="pmrow")
                nc.sync.dma_start(pm_row[:], pmask_d[s, None, :])
                return X, cTb, prc, pm_row

            def conv_pair(XA, XB):
                """3 conv layers on a sample pair; returns (psTA, psTB)."""
                for i in range(3):
                    last = i == 2
                    if last:
                        oA = pp.tile([D, L], F16, tag="psT")
                        oB = pp.tile([D, L], F16, tag="psT")
                    else:
                        oA = xp.tile([128, XW], F16, tag="X")
                        oB = xp.tile([128, XW], F16, tag="X")
                        for o in (oA, oB):
                            nc.gpsimd.memset(o[0:D, 0:LP], 0.0)
                            nc.gpsimd.memset(o[0:D, LP + L:XW], 0.0)
                            nc.gpsimd.memset(o[D:128, 0:LP - 1], 0.0)
                            nc.gpsimd.memset(o[D:128, LP - 1 + L:XW], 0.0)
                    for b in range(4):
                        pv = pc.tile([128, 512], F32, tag="cv")
                        for j in range(12):
                            w = tk_sb[:, (i * 12 + j) * D:(i * 12 + j + 1) * D]
                            st, sp = j == 0, j == 11
                            c0 = 2 * j + b * 512 + LP - PADK
                            nc.tensor.matmul(pv[0:D, :], w, XA[:, c0:c0 + 512],
                                             start=st, stop=sp, skip_group_check=True)
                            nc.tensor.matmul(pv[D:128, :], w, XB[:, c0:c0 + 512],
                                             start=st, stop=sp, skip_group_check=True)
                        bl = b * 512
                        if last:
                            nc.scalar.activation(oA[:, bl:bl + 512], pv[0:D, :],
                                                 AF.Relu, bias=cb_sb[0:D, i:i + 1])
                            nc.vector.tensor_scalar(
                                oB[:, bl:bl + 512], pv[D:128, :],
                                cb_sb[D:128, i:i + 1], 0.0, op0=OP.add, op1=OP.max)
                        else:
                            # A: relu on ACT, then fp16 shifted copy on DVE
                            nc.scalar.activation(
                                oA[0:D, LP + bl: LP + bl + 512], pv[0:D, :],
                                AF.Relu, bias=cb_sb[0:D, i:i + 1])
                            nc.vector.tensor_copy(
                                oA[D:128, LP - 1 + bl: LP - 1 + bl + 512],
                                oA[0:D, LP + bl: LP + bl + 512])
                            # B: relu on DVE, then fp16 shifted copy on DVE
                            nc.vector.tensor_scalar(
                                oB[0:D, LP + bl: LP + bl + 512], pv[D:128, :],
                                cb_sb[D:128, i:i + 1], 0.0, op0=OP.add, op1=OP.max)
                            nc.vector.tensor_copy(
                                oB[D:128, LP - 1 + bl: LP - 1 + bl + 512],
                                oB[0:D, LP + bl: LP + bl + 512])
                    if not last:
                        XA, XB = oA, oB
                return oA, oB

            def attention(s, psT, cTb, prc, pm_row, sub=99):
                hsT = pp.tile([D, L], F16, tag="hsT")
                for b in range(4):
                    ph = pa.tile([128, 512], F32, tag="at")
                    nc.tensor.matmul(ph[0:D, :], wa_sb[0:D, :],
                                     psT[:, b * 512:(b + 1) * 512],
                                     start=True, stop=True)
                    nc.scalar.activation(hsT[:, b * 512:(b + 1) * 512], ph[0:D, :],
                                         AF.Relu, bias=ba_sb[0:D, :])
                if sub < 2:
                    return
                pq = pz.tile([128, 512], F32, tag="ss")
                nc.tensor.matmul(pq[0:D, 0:1], wa_sb[0:D, :], cTb[:],
                                 start=True, stop=True)
                hq = sm.tile([D, 1], F16, tag="hq")
                nc.scalar.activation(hq[:], pq[0:D, 0:1], AF.Relu,
                                     bias=ba_sb[0:D, :])
                pacc = sm.tile([D, 4], F32, tag="pacc")
                for b in range(4):
                    bl = b * 512
                    if sub < 3:
                        continue
                    pw = pa.tile([128, 512], F32, tag="at")
                    nc.tensor.matmul(pw[0:1, :], hq[:], hsT[:, bl:bl + 512],
                                     start=True, stop=True)
                    if sub < 4:
                        continue
                    # mask before tanh (exact for 0/1 masks; tanh(0)=0)
                    wrm = sm.tile([1, 512], F16, tag="wrm")
                    nc.vector.tensor_tensor(wrm[:], pw[0:1, :], pm_row[:, bl:bl + 512],
                                            op=OP.mult)
                    if sub < 5:
                        continue
                    pwb = pa.tile([128, 512], F32, tag="at")
                    nc.tensor.matmul(pwb[0:D, :], ones_rh[:], wrm[:],
                                     start=True, stop=True)
                    if sub < 6:
                        continue
                    wbb = sm.tile([D, 512], F16, tag="wbb")
                    nc.scalar.activation(wbb[:], pwb[0:D, :], AF.Tanh)
                    if sub < 7:
                        continue
                    scr = sm.tile([D, 512], F16, tag="scr")
                    nc.vector.scalar_tensor_tensor(
                        scr[:], hsT[:, bl:bl + 512], 1.0, wbb[:],
                        op0=OP.mult, op1=OP.mult, accum_out=pacc[:, b:b + 1])
                if sub < 8:
                    return
                pj = sm.tile([D, 4], F32, tag="pj")
                praw = sm.tile([D, 1], F32, tag="praw")
                nc.scalar.activation(pj[:], pacc[:], AF.Copy, accum_out=praw[:])
                nc.vector.tensor_tensor(catC[D:128, s:s + 1], praw[:],
                                        prc[:], op=OP.mult)

            # ================= main loop =================
            for t in range(nsamp // 2):
                s0, s1 = 2 * t, 2 * t + 1
                XA, cTb0, prc0, pmr0 = prep_sample(s0)
                XB, cTb1, prc1, pmr1 = prep_sample(s1)
                if stage < 2:
                    if t == 0:
                        nc.scalar.copy(catC[D:128, 0:2], XA[0:D, 100:102])
                    continue
                psTA, psTB = conv_pair(XA, XB)
                if stage < 3:
                    if t == 0:
                        nc.scalar.copy(catC[D:D + 2, 0:2], psTA[0:2, 0:2])
                    continue
                attention(s0, psTA, cTb0, prc0, pmr0, sub)
                attention(s1, psTB, cTb1, prc1, pmr1, sub)

            # ================= output MLP =================
            p1 = pz.tile([128, 512], F32, tag="ss")
            nc.tensor.matmul(p1[0:128, 0:nsamp], wo_sb[:, 0:128], catC[:],
                             start=True, stop=True)
            cat1 = sm.tile([128, nsamp], F32, tag="cat1")
            nc.scalar.activation(cat1[:], p1[0:128, 0:nsamp], AF.Relu,
                                 bias=bo_sb[:, 0:1])
            p2 = pz.tile([128, 512], F32, tag="ss")
            nc.tensor.matmul(p2[0:128, 0:nsamp], wo_sb[:, 128:256], cat1[:],
                             start=True, stop=True)
            cat2 = sm.tile([128, nsamp], F32, tag="cat2")
            nc.scalar.activation(cat2[:], p2[0:128, 0:nsamp], AF.Relu,
                                 bias=bo_sb[:, 1:2])
            p3 = pz.tile([128, 512], F32, tag="ss")
            nc.tensor.matmul(p3[0:2, 0:nsamp], wi_sb[:], cat2[:],
                             start=True, stop=True)
            outS = sm.tile([2, nsamp], F32, tag="os")
            nc.scalar.activation(outS[:], p3[0:2, 0:nsamp], AF.Identity,
                                 bias=bi_sb[:])
            nc.sync.dma_start(out_d[:], outS[:])

    nc.compile()
    return nc


PADK = 11  # conv kernel half-width


def build_tk(conv_k):
    """conv_k [3, 23, 23] -> TK [3, 12, 128, 64] banded matrices.
    TK[i][j][(s, d_in), d_out] = conv_k[i, 2j+s, d_in - d_out + 11]."""
    TK = np.zeros((3, 12, 128, D), np.float32)
    ck = np.asarray(conv_k, np.float32)
    for i in range(3):
        for kh in range(23):
            j, sl = divmod(kh, 2)
            for do in range(D):
                lo = max(0, do - PADK)
                hi = min(D, do + PADK + 1)
                TK[i, j, sl * D + lo: sl * D + hi, do] = \
                    ck[i, kh, lo - do + PADK: hi - do + PADK]
    return TK


def make_in_maps(inputs, nsamp=NS, ncores=NCORES):
    f32 = lambda x: np.ascontiguousarray(np.asarray(x), dtype=np.float32)
    f16 = lambda x: np.ascontiguousarray(np.asarray(x), dtype=np.float16)

    wg = f16(np.concatenate(
        [np.transpose(f32(inputs["W_gnn"]), (0, 2, 1)),
         f32(inputs["b_gnn"])[:, None, :]], axis=1))           # [3, 65, 64]
    tk = f16(build_tk(inputs["conv_k"]))
    cb = np.repeat(f32(inputs["conv_b"])[:, None], 128, axis=1)  # [3, 128]
    waT = f32(inputs["W_att"]).T                                 # [64, 64]
    wa = f16(np.concatenate([waT, waT], axis=0))                 # [128, 64]
    ba = np.concatenate([f32(inputs["b_att"])] * 2)              # [128]
    wo = np.ascontiguousarray(np.transpose(f32(inputs["W_out"]), (0, 2, 1)))
    wi = np.ascontiguousarray(f32(inputs["W_int"]).T)            # [128, 2]

    B = np.asarray(inputs["amino"]).shape[0]
    # layer-0 conv image: gathered word embeddings, transposed, padded,
    # with the one-column-shifted copy in partition rows 64..127
    embw16 = f16(inputs["emb_word"])                             # [10000, 64]
    gT = embw16[np.asarray(inputs["amino"])].transpose(0, 2, 1)  # [B, 64, L]
    ximg = np.zeros((B, 128, XW), np.float16)
    ximg[:, 0:D, LP:LP + L] = gT
    ximg[:, D:128, LP - 1:LP - 1 + L] = gT

    # initial GNN state, transposed, with bias row of ones
    embf = f32(inputs["emb_fp"])                                 # [2000, 64]
    g0 = embf[np.asarray(inputs["atoms"])].transpose(0, 2, 1)    # [B, 64, N]
    xst0 = np.ones((B, D + 1, N), np.float32)
    xst0[:, 0:D, :] = g0

    shared = dict(wg=wg, tk=tk, cb=cb, wa=wa, ba=ba, wo=wo,
                  bo=f32(inputs["b_out"]), wi=wi, bi=f32(inputs["b_int"]))
    amask = f32(inputs["atoms_mask"])
    pmask = f32(inputs["amino_mask"])
    adjt = f16(np.transpose(f32(inputs["adjacency"]), (0, 2, 1)))

    in_maps = []
    for c in range(ncores):
        sl = slice(c * nsamp, (c + 1) * nsamp)
        m = dict(shared)
        m.update(ximg=ximg[sl], xst0=xst0[sl], amask=amask[sl],
                 pmask=pmask[sl], adjt=adjt[sl])
        in_maps.append(m)
    return in_maps


_NC_CACHE = {}


def _get_nc(nsamp=NS):
    if nsamp not in _NC_CACHE:
        _NC_CACHE[nsamp] = build_nc(nsamp)
    return _NC_CACHE[nsamp]


def kernel(**inputs):
    nc = _get_nc(NS)
    in_maps = make_in_maps(inputs, NS, NCORES)
    res = run_bass_kernel_spmd(nc, in_maps, core_ids=list(range(NCORES)))
    out = np.concatenate([np.asarray(r["out"]).T for r in res.results], axis=0)
    return np.ascontiguousarray(out, dtype=np.float32)


# revision 18
# speedup vs baseline: 1.0598x; 1.0598x over previous
"""Self-contained Trainium2 Bass kernel for nn_CPINet_36850819400255.

Strategy: pure data parallelism over batch B=256 -> 8 cores x 32 samples.
Per core the dominant cost is the 3-layer 23x23 conv over [2048, 64] maps,
computed as 12 accumulating K=128 fp16 matmuls per 512-col block against a
transposed, zero-padded image whose partition rows 64..127 hold a copy
shifted by one column (each matmul covers two kernel rows).  Two samples
run concurrently in PE column groups 0-63 / 64-127.  Embedding gathers and
the layer-0 image transpose are host-side input transforms; the device
sees ready-made [128, 2080] fp16 images.  GNN keeps its state transposed:
px^T = (A @ hs)^T is computed as lhsT=hs, rhs=A^T (no PE transposes).
"""

import sys

sys.path.insert(0, "/opt/trn_rl_repo")

import numpy as np

import concourse.bass as bass
import concourse.mybir as mybir
import concourse.tile as tile
from concourse import bacc
from concourse.bass_utils import run_bass_kernel_spmd

F32 = mybir.dt.float32
F16 = mybir.dt.float16
AF = mybir.ActivationFunctionType
OP = mybir.AluOpType

NCORES = 8
B_TOT = 256
NS = B_TOT // NCORES          # samples per core
N = 128                       # atoms
L = 2048                      # amino length
D = 64
LP = 16                       # left pad of the conv image (32B aligned)
XW = 2080                     # padded width of transposed conv image
EPS = 1e-6


def build_nc(nsamp=NS, stage=99, sub=99):
    """stage: 1=prep only, 2=+conv, 3=+attention, 99=full."""
    nc = bacc.Bacc("TRN2", target_bir_lowering=False, debug=True)

    # ---- DRAM I/O ----
    ximg_d = nc.dram_tensor("ximg", [nsamp, 128, XW], F16, kind="ExternalInput")
    xst0_d = nc.dram_tensor("xst0", [nsamp, D + 1, N], F32, kind="ExternalInput")
    amask_d = nc.dram_tensor("amask", [nsamp, N], F32, kind="ExternalInput")
    pmask_d = nc.dram_tensor("pmask", [nsamp, L], F32, kind="ExternalInput")
    adjt_d = nc.dram_tensor("adjt", [nsamp, N, N], F16, kind="ExternalInput")
    wg_d = nc.dram_tensor("wg", [3, D + 1, D], F16, kind="ExternalInput")
    tk_d = nc.dram_tensor("tk", [3, 12, 128, D], F16, kind="ExternalInput")
    cb_d = nc.dram_tensor("cb", [3, 128], F32, kind="ExternalInput")
    wa_d = nc.dram_tensor("wa", [128, D], F16, kind="ExternalInput")
    ba_d = nc.dram_tensor("ba", [128], F32, kind="ExternalInput")
    wo_d = nc.dram_tensor("wo", [2, 128, 128], F32, kind="ExternalInput")
    bo_d = nc.dram_tensor("bo", [2, 128], F32, kind="ExternalInput")
    wi_d = nc.dram_tensor("wi", [128, 2], F32, kind="ExternalInput")
    bi_d = nc.dram_tensor("bi", [2], F32, kind="ExternalInput")
    out_d = nc.dram_tensor("out", [2, nsamp], F32, kind="ExternalOutput")

    with tile.TileContext(nc) as tc:
        with (
            tc.tile_pool(name="cp", bufs=1) as cp,          # constants
            tc.tile_pool(name="xp", bufs=8) as xp,          # conv images
            tc.tile_pool(name="pp", bufs=3) as pp,          # psT / hsT
            tc.tile_pool(name="sm", bufs=3) as sm,          # small sbuf
            tc.tile_pool(name="pc", bufs=3, space="PSUM") as pc,   # conv psum
            tc.tile_pool(name="pa", bufs=3, space="PSUM") as pa,   # attn psum
            tc.tile_pool(name="pz", bufs=2, space="PSUM") as pz,   # small psum
        ):
            # ---------- constants ----------
            ones_c = cp.tile([128, D], F32, tag="ones_c")
            nc.vector.memset(ones_c[:], 1.0)
            ones65 = cp.tile([1, D + 1], F32, tag="ones65")
            nc.vector.memset(ones65[:], 1.0)
            ones_rh = cp.tile([1, D], F16, tag="ones_rh")
            nc.vector.memset(ones_rh[:], 1.0)

            tk_sb = cp.tile([128, 3 * 12 * D], F16, tag="tk")
            for i in range(3):
                for j in range(12):
                    k = i * 12 + j
                    nc.sync.dma_start(tk_sb[:, k * D:(k + 1) * D], tk_d[i, j])
            wg_sb = cp.tile([D + 1, 3 * D], F16, tag="wg")
            for i in range(3):
                nc.sync.dma_start(wg_sb[:, i * D:(i + 1) * D], wg_d[i])
            wa_sb = cp.tile([128, D], F16, tag="wa")
            nc.sync.dma_start(wa_sb[:], wa_d[:])
            ba_sb = cp.tile([128, 1], F32, tag="ba")
            nc.sync.dma_start(ba_sb[:], ba_d[:, None])
            cb_sb = cp.tile([128, 3], F32, tag="cb")
            for i in range(3):
                nc.sync.dma_start(cb_sb[:, i:i + 1], cb_d[i, :, None])
            wo_sb = cp.tile([128, 256], F32, tag="wo")
            for j in range(2):
                nc.sync.dma_start(wo_sb[:, j * 128:(j + 1) * 128], wo_d[j])
            bo_sb = cp.tile([128, 2], F32, tag="bo")
            for j in range(2):
                nc.sync.dma_start(bo_sb[:, j:j + 1], bo_d[j, :, None])
            wi_sb = cp.tile([128, 2], F32, tag="wi")
            nc.sync.dma_start(wi_sb[:], wi_d[:])
            bi_sb = cp.tile([2, 1], F32, tag="bi")
            nc.sync.dma_start(bi_sb[:], bi_d[:, None])

            catC = cp.tile([128, nsamp], F32, tag="cat")

            def prep_sample(s):
                """DMA loads + GNN + compound for sample s.
                Returns (X, cTb, prc, pm_row)."""
                X = xp.tile([128, XW], F16, tag="X")
                nc.sync.dma_start(X[:], ximg_d[s])

                am_col = sm.tile([N, 1], F32, tag="amcol")
                nc.sync.dma_start(am_col[:], amask_d[s, :, None])
                am_row = sm.tile([1, N], F32, tag="amrow")
                nc.sync.dma_start(am_row[:], amask_d[s, None, :])
                ATh = sm.tile([N, N], F16, tag="AT")
                nc.sync.dma_start(ATh[:], adjt_d[s])

                # ---- GNN (state kept transposed: xsT [65, 128] fp32) ----
                xsT = sm.tile([D + 1, N], F32, tag="xst")
                nc.sync.dma_start(xsT[:], xst0_d[s])
                for i in range(3):
                    xsTb = sm.tile([D + 1, N], F16, tag="xstb")
                    nc.vector.tensor_copy(xsTb[:], xsT[:])
                    ph = pz.tile([128, 512], F32, tag="ss")
                    nc.tensor.matmul(ph[0:N, 0:D], xsTb[:],
                                     wg_sb[:, i * D:(i + 1) * D],
                                     start=True, stop=True)
                    hs = sm.tile([N, D], F16, tag="hs")
                    nc.scalar.activation(hs[:], ph[0:N, 0:D], AF.Relu)
                    px = pz.tile([128, 512], F32, tag="ss")
                    nc.tensor.matmul(px[0:D, 0:N], hs[:], ATh[:],
                                     start=True, stop=True)
                    xsT2 = sm.tile([D + 1, N], F32, tag="xst")
                    nc.vector.memset(xsT2[D:D + 1, :], 1.0)
                    nc.vector.tensor_add(xsT2[0:D, :], px[0:D, 0:N], xsT[0:D, :])
                    xsT = xsT2

                # ---- compound (masked mean over atoms) ----
                amb = pz.tile([128, 512], F32, tag="ss")
                nc.tensor.matmul(amb[0:D + 1, 0:N], ones65[:], am_row[:],
                                 start=True, stop=True)
                scrc = sm.tile([D, N], F32, tag="scrc")
                cm = sm.tile([D, 1], F32, tag="cm")
                nc.vector.scalar_tensor_tensor(
                    scrc[:], xsT[0:D, :], 1.0, amb[0:D, 0:N],
                    op0=OP.mult, op1=OP.mult, accum_out=cm[:])
                pd = pz.tile([128, 512], F32, tag="ss")
                nc.tensor.matmul(pd[0:D, 0:1], ones_c[:], am_col[:],
                                 start=True, stop=True)
                dn = sm.tile([D, 1], F32, tag="dn")
                nc.vector.tensor_scalar_add(dn[:], pd[0:D, 0:1], EPS)
                rc = sm.tile([D, 1], F32, tag="rc")
                nc.vector.reciprocal(rc[:], dn[:])
                cT = sm.tile([D, 1], F32, tag="ct")
                nc.vector.tensor_tensor(cT[:], cm[:], rc[:], op=OP.mult)
                nc.vector.tensor_copy(catC[0:D, s:s + 1], cT[:])
                cTb = sm.tile([D, 1], F16, tag="ctb")
                nc.vector.tensor_copy(cTb[:], cT[:])

                # ---- protein mask ----
                pm16 = sm.tile([128, 16], F32, tag="pm16")
                nc.sync.dma_start(pm16[:], pmask_d[s].rearrange("(p t) -> p t", t=16))
                pmj = sm.tile([128, 16], F32, tag="pmj")
                pmsum = sm.tile([128, 1], F32, tag="pmsum")
                nc.scalar.activation(pmj[:], pm16[:], AF.Copy, accum_out=pmsum[:])
                ppd = pz.tile([128, 512], F32, tag="ss")
                nc.tensor.matmul(ppd[0:D, 0:1], ones_c[:], pmsum[:], start=True, stop=True)
                pdn = sm.tile([D, 1], F32, tag="pdn")
                nc.vector.tensor_scalar_add(pdn[:], ppd[0:D, 0:1], EPS)
                prc = sm.tile([D, 1], F32, tag="prc")
                nc.vector.reciprocal(prc[:], pdn[:])
                pm_row = sm.tile([1, L], F32, tag="pmrow")
                nc.sync.dma_start(pm_row[:], pmask_d[s, None, :])
                return X, cTb, prc, pm_row

            def conv_pair(XA, XB):
                """3 conv layers on a sample pair; returns (psTA, psTB)."""
                for i in range(3):
                    last = i == 2
                    if last:
                        oA = pp.tile([D, L], F16, tag="psT")
                        oB = pp.tile([D, L], F16, tag="psT")
                    else:
                        oA = xp.tile([128, XW], F16, tag="X")
                        oB = xp.tile([128, XW], F16, tag="X")
                        for o in (oA, oB):
                            nc.gpsimd.memset(o[0:D, 0:LP], 0.0)
                            nc.gpsimd.memset(o[0:D, LP + L:XW], 0.0)
                            nc.gpsimd.memset(o[D:128, 0:LP - 1], 0.0)
                            nc.gpsimd.memset(o[D:128, LP - 1 + L:XW], 0.0)
                    for b in range(4):
                        pv = pc.tile([128, 512], F32, tag="cv")
                        for j in range(12):
                            w = tk_sb[:, (i * 12 + j) * D:(i * 12 + j + 1) * D]
                            st, sp = j == 0, j == 11
                            c0 = 2 * j + b * 512 + LP - PADK
                            nc.tensor.matmul(pv[0:D, :], w, XA[:, c0:c0 + 512],
                                             start=st, stop=sp, skip_group_check=True)
                            nc.tensor.matmul(pv[D:128, :], w, XB[:, c0:c0 + 512],
                                             start=st, stop=sp, skip_group_check=True)
                        bl = b * 512
                        if last:
                            nc.scalar.activation(oA[:, bl:bl + 512], pv[0:D, :],
                                                 AF.Relu, bias=cb_sb[0:D, i:i + 1])
                            nc.vector.tensor_scalar(
                                oB[:, bl:bl + 512], pv[D:128, :],
                                cb_sb[D:128, i:i + 1], 0.0, op0=OP.add, op1=OP.max)
                        else:
                            # A: relu on ACT, then fp16 shifted copy on DVE
                            nc.scalar.activation(
                                oA[0:D, LP + bl: LP + bl + 512], pv[0:D, :],
                                AF.Relu, bias=cb_sb[0:D, i:i + 1])
                            nc.vector.tensor_copy(
                                oA[D:128, LP - 1 + bl: LP - 1 + bl + 512],
                                oA[0:D, LP + bl: LP + bl + 512])
                            # B: relu on DVE, then fp16 shifted copy on DVE
                            nc.vector.tensor_scalar(
                                oB[0:D, LP + bl: LP + bl + 512], pv[D:128, :],
                                cb_sb[D:128, i:i + 1], 0.0, op0=OP.add, op1=OP.max)
                            nc.vector.tensor_copy(
                                oB[D:128, LP - 1 + bl: LP - 1 + bl + 512],
                                oB[0:D, LP + bl: LP + bl + 512])
                    if not last:
                        XA, XB = oA, oB
                return oA, oB

            def attention(s, psT, cTb, prc, pm_row, sub=99):
                hsT = pp.tile([D, L], F16, tag="hsT")
                for b in range(4):
                    ph = pa.tile([128, 512], F32, tag="at")
                    nc.tensor.matmul(ph[0:D, :], wa_sb[0:D, :],
                                     psT[:, b * 512:(b + 1) * 512],
                                     start=True, stop=True)
                    nc.scalar.activation(hsT[:, b * 512:(b + 1) * 512], ph[0:D, :],
                                         AF.Relu, bias=ba_sb[0:D, :])
                if sub < 2:
                    return
                pq = pz.tile([128, 512], F32, tag="ss")
                nc.tensor.matmul(pq[0:D, 0:1], wa_sb[0:D, :], cTb[:],
                                 start=True, stop=True)
                hq = sm.tile([D, 1], F16, tag="hq")
                nc.scalar.activation(hq[:], pq[0:D, 0:1], AF.Relu,
                                     bias=ba_sb[0:D, :])
                pacc = sm.tile([D, 4], F32, tag="pacc")
                for b in range(4):
                    bl = b * 512
                    if sub < 3:
                        continue
                    pw = pa.tile([128, 512], F32, tag="at")
                    nc.tensor.matmul(pw[0:1, :], hq[:], hsT[:, bl:bl + 512],
                                     start=True, stop=True)
                    if sub < 4:
                        continue
                    # mask before tanh (exact for 0/1 masks; tanh(0)=0)
                    wrm = sm.tile([1, 512], F16, tag="wrm")
                    nc.vector.tensor_tensor(wrm[:], pw[0:1, :], pm_row[:, bl:bl + 512],
                                            op=OP.mult)
                    if sub < 5:
                        continue
                    pwb = pa.tile([128, 512], F32, tag="at")
                    nc.tensor.matmul(pwb[0:D, :], ones_rh[:], wrm[:],
                                     start=True, stop=True)
                    if sub < 6:
                        continue
                    wbb = sm.tile([D, 512], F16, tag="wbb")
                    nc.scalar.activation(wbb[:], pwb[0:D, :], AF.Tanh)
                    if sub < 7:
                        continue
                    scr = sm.tile([D, 512], F16, tag="scr")
                    nc.vector.scalar_tensor_tensor(
                        scr[:], hsT[:, bl:bl + 512], 1.0, wbb[:],
                        op0=OP.mult, op1=OP.mult, accum_out=pacc[:, b:b + 1])
                if sub < 8:
                    return
                pj = sm.tile([D, 4], F32, tag="pj")
                praw = sm.tile([D, 1], F32, tag="praw")
                nc.scalar.activation(pj[:], pacc[:], AF.Copy, accum_out=praw[:])
                nc.vector.tensor_tensor(catC[D:128, s:s + 1], praw[:],
                                        prc[:], op=OP.mult)

            # ================= main loop =================
            for t in range(nsamp // 2):
                s0, s1 = 2 * t, 2 * t + 1
                XA, cTb0, prc0, pmr0 = prep_sample(s0)
                XB, cTb1, prc1, pmr1 = prep_sample(s1)
                if stage < 2:
                    if t == 0:
                        nc.scalar.copy(catC[D:128, 0:2], XA[0:D, 100:102])
                    continue
                psTA, psTB = conv_pair(XA, XB)
                if stage < 3:
                    if t == 0:
                        nc.scalar.copy(catC[D:D + 2, 0:2], psTA[0:2, 0:2])
                    continue
                attention(s0, psTA, cTb0, prc0, pmr0, sub)
                attention(s1, psTB, cTb1, prc1, pmr1, sub)

            # ================= output MLP =================
            p1 = pz.tile([128, 512], F32, tag="ss")
            nc.tensor.matmul(p1[0:128, 0:nsamp], wo_sb[:, 0:128], catC[:],
                             start=True, stop=True)
            cat1 = sm.tile([128, nsamp], F32, tag="cat1")
            nc.scalar.activation(cat1[:], p1[0:128, 0:nsamp], AF.Relu,
                                 bias=bo_sb[:, 0:1])
            p2 = pz.tile([128, 512], F32, tag="ss")
            nc.tensor.matmul(p2[0:128, 0:nsamp], wo_sb[:, 128:256], cat1[:],
                             start=True, stop=True)
            cat2 = sm.tile([128, nsamp], F32, tag="cat2")
            nc.scalar.activation(cat2[:], p2[0:128, 0:nsamp], AF.Relu,
                                 bias=bo_sb[:, 1:2])
            p3 = pz.tile([128, 512], F32, tag="ss")
            nc.tensor.matmul(p3[0:2, 0:nsamp], wi_sb[:], cat2[:],
                             start=True, stop=True)
            outS = sm.tile([2, nsamp], F32, tag="os")
            nc.scalar.activation(outS[:], p3[0:2, 0:nsamp], AF.Identity,
                                 bias=bi_sb[:])
            nc.sync.dma_start(out_d[:], outS[:])

    nc.compile()
    return nc


PADK = 11  # conv kernel half-width


def build_tk(conv_k):
    """conv_k [3, 23, 23] -> TK [3, 12, 128, 64] banded matrices.
    TK[i][j][(s, d_in), d_out] = conv_k[i, 2j+s, d_in - d_out + 11]."""
    TK = np.zeros((3, 12, 128, D), np.float32)
    ck = np.asarray(conv_k, np.float32)
    for i in range(3):
        for kh in range(23):
            j, sl = divmod(kh, 2)
            for do in range(D):
                lo = max(0, do - PADK)
                hi = min(D, do + PADK + 1)
                TK[i, j, sl * D + lo: sl * D + hi, do] = \
                    ck[i, kh, lo - do + PADK: hi - do + PADK]
    return TK


def make_in_maps(inputs, nsamp=NS, ncores=NCORES):
    f32 = lambda x: np.ascontiguousarray(np.asarray(x), dtype=np.float32)
    f16 = lambda x: np.ascontiguousarray(np.asarray(x), dtype=np.float16)

    wg = f16(np.concatenate(
        [np.transpose(f32(inputs["W_gnn"]), (0, 2, 1)),
         f32(inputs["b_gnn"])[:, None, :]], axis=1))           # [3, 65, 64]
    tk = f16(build_tk(inputs["conv_k"]))
    cb = np.repeat(f32(inputs["conv_b"])[:, None], 128, axis=1)  # [3, 128]
    waT = f32(inputs["W_att"]).T                                 # [64, 64]
    wa = f16(np.concatenate([waT, waT], axis=0))                 # [128, 64]
    ba = np.concatenate([f32(inputs["b_att"])] * 2)              # [128]
    wo = np.ascontiguousarray(np.transpose(f32(inputs["W_out"]), (0, 2, 1)))
    wi = np.ascontiguousarray(f32(inputs["W_int"]).T)            # [128, 2]

    B = np.asarray(inputs["amino"]).shape[0]
    # layer-0 conv image: gathered word embeddings, transposed, padded,
    # with the one-column-shifted copy in partition rows 64..127
    embw16 = f16(inputs["emb_word"])                             # [10000, 64]
    gT = embw16[np.asarray(inputs["amino"])].transpose(0, 2, 1)  # [B, 64, L]
    ximg = np.zeros((B, 128, XW), np.float16)
    ximg[:, 0:D, LP:LP + L] = gT
    ximg[:, D:128, LP - 1:LP - 1 + L] = gT

    # initial GNN state, transposed, with bias row of ones
    embf = f32(inputs["emb_fp"])                                 # [2000, 64]
    g0 = embf[np.asarray(inputs["atoms"])].transpose(0, 2, 1)    # [B, 64, N]
    xst0 = np.ones((B, D + 1, N), np.float32)
    xst0[:, 0:D, :] = g0

    shared = dict(wg=wg, tk=tk, cb=cb, wa=wa, ba=ba, wo=wo,
                  bo=f32(inputs["b_out"]), wi=wi, bi=f32(inputs["b_int"]))
    amask = f32(inputs["atoms_mask"])
    pmask = f32(inputs["amino_mask"])
    adjt = f16(np.transpose(f32(inputs["adjacency"]), (0, 2, 1)))

    in_maps = []
    for c in range(ncores):
        sl = slice(c * nsamp, (c + 1) * nsamp)
        m = dict(shared)
        m.update(ximg=ximg[sl], xst0=xst0[sl], amask=amask[sl],
                 pmask=pmask[sl], adjt=adjt[sl])
        in_maps.append(m)
    return in_maps


_NC_CACHE = {}


def _get_nc(nsamp=NS):
    if nsamp not in _NC_CACHE:
        _NC_CACHE[nsamp] = build_nc(nsamp)
    return _NC_CACHE[nsamp]


def kernel(**inputs):
    nc = _get_nc(NS)
    in_maps = make_in_maps(inputs, NS, NCORES)
    res = run_bass_kernel_spmd(nc, in_maps, core_ids=list(range(NCORES)))
    out = np.concatenate([np.asarray(r["out"]).T for r in res.results], axis=0)
    return np.ascontiguousarray(out, dtype=np.float32)
